# revision 1
# baseline (speedup 1.0000x reference)
"""Trainium2 Bass kernel for nn_CrossAttention_55130200212194.

Sharding: head h -> core h (8 heads, 8 cores, one replicated NEFF; cores
differ only in input data).  Inputs are re-laid-out on the host (transposes /
per-head slices = DRAM layout prep for the replicated tensors); every FLOP of
the module (3 score GEMMs, 2 softmaxes, 2 attn@v GEMMs, q/v projections,
output projection + bias) runs on device.

All matmul operands are float32r (single-pass fp32 matmul mode, 1 PE
cycle/row vs 4 for plain fp32; measured end-to-end rel err vs the fp32
reference: 3e-4).  PSUM accumulation is fp32.

Per-core device pipeline (scores kept transposed, [kv j, query i]):
  qcT  = Wq_h @ x.T                (K=640 over 5 c-tiles)    [80,2048]
  vself= x @ Wv_h.T                                          [2048,80]
  for i-chunk (4 x 512 cols), j-tile (16 x 128 rows):
      ps_m = klT.T@qiT + krT.T@(qcT*(0.3/0.7))   (PSUM accumulate, K=128
                                                  zero-padded from 80)
      ps_s = kiT.T@qiT
      em = exp(0.7*SCALE*ps_m); es = exp(SCALE*ps_s)    (ACT, free affine)
      outD += v_ref_ext[j].T @ em     # v extended with ones cols 80:128 ->
      outS += v_self_ext[j].T @ es    # Z=sum(exp) lands on psum rows 80:128
  blend (deferred one chunk to keep the PE queue busy):
      mergedT[:, chunk] = 0.7*outD/Z_D + 0.3*outS/Z_S
      (1/Z via DVE reciprocal on the 32-aligned rows [96:128]; broadcast of
       [1,512] across partitions via a K=128 one-hot matmul)
      then project: out[n-tile] = mergedT_h.T @ Wout[:,h-cols].T + bias(core0)
Host: sum of the 8 partial [2048,640] projections -> [1, 2048, 640]
(column-sharded tensor-parallel Wout with the reduce done on host).
"""

import os
import sys

sys.path.insert(0, "/opt/trn_rl_repo")

import numpy as np

H = 8
N = 2048
D = 80
C = 640
SCALE = D ** -0.5
GAMMA = 0.7  # dual-path logit mix (and 1-BETA blend weight)
BETA = 0.3
P = 128
IC = 512                 # i-chunk (PSUM bank = 512 fp32)
NJT = N // P             # 16 j-tiles
NICH = N // IC           # 4 i-chunks
NCT = C // P             # 5 c-tiles
NNT = N // P             # 16 n-tiles
NCORES = 8

_CACHE = {}
LAST_EXEC_NS = None


def _build_nc():
    import concourse.mybir as mybir
    import concourse.tile as tile
    from concourse import bacc
    from concourse.bass import ts

    f32 = mybir.dt.float32
    f32r = mybir.dt.float32r
    Exp = mybir.ActivationFunctionType.Exp

    nc = bacc.Bacc(
        "TRN2",
        target_bir_lowering=False,
        debug=False,
        enable_asserts=False,
        num_devices=NCORES,
    )

    # fp32 matmuls run at 1/4 PE rate (two half-speed passes); float32r is the
    # single-pass mode (same 4-byte layout).  The BIR verifier requires every
    # producer of an f32r-matmul operand to emit f32r, so all tensors on the
    # matmul paths are declared float32r end to end.
    xT_d = nc.dram_tensor("xT", [C, N], f32r, kind="ExternalInput")
    qiT_d = nc.dram_tensor("qiT", [D, N], f32r, kind="ExternalInput")
    kiT_d = nc.dram_tensor("kiT", [D, N], f32r, kind="ExternalInput")
    krT_d = nc.dram_tensor("krT", [D, N], f32r, kind="ExternalInput")
    klT_d = nc.dram_tensor("klT", [D, N], f32r, kind="ExternalInput")
    vref_d = nc.dram_tensor("vref", [N, D], f32r, kind="ExternalInput")
    WqhT_d = nc.dram_tensor("WqhT", [C, D], f32r, kind="ExternalInput")
    WvhT_d = nc.dram_tensor("WvhT", [C, D], f32r, kind="ExternalInput")
    WoT_d = nc.dram_tensor("WoT", [D, C], f32r, kind="ExternalInput")
    bias_d = nc.dram_tensor("bias", [C], f32r, kind="ExternalInput")
    # blend-weight constant: row 96 of block 0 = GAMMA (picks 1/Z_D), row 96
    # of block 1 = BETA (picks 1/Z_S); everything else zero
    blendw_d = nc.dram_tensor("blendw", [P, 2 * D], f32r, kind="ExternalInput")
    out_d = nc.dram_tensor("out", [N, C], f32, kind="ExternalOutput")

    with tile.TileContext(nc) as tc:
        with (
            tc.tile_pool(name="const", bufs=1) as const,
            tc.tile_pool(name="work", bufs=3) as work,
            tc.tile_pool(name="fout", bufs=3) as fout,
        ):
            # ---- persistent SBUF tiles ----
            xT = const.tile([P, NCT, N], f32r, tag="xT")
            qiT = const.tile([P, N], f32r, tag="qiT")
            kiT = const.tile([P, N], f32r, tag="kiT")
            krT = const.tile([P, N], f32r, tag="krT")
            klT = const.tile([P, N], f32r, tag="klT")
            qcT = const.tile([P, N], f32r, tag="qcT")
            WqhT = const.tile([P, NCT, D], f32r, tag="WqhT")
            WvhT = const.tile([P, NCT, 256], f32r, tag="WvhT")
            # v extended to 128 columns: cols 0:80 = v, cols 80:128 = 1.0.
            # The attn@v matmul then yields sum(exp) == Z duplicated on psum
            # partitions 80..127, so the 32-aligned slice [96:128] can be
            # copied/reciprocal'd (partition starts must be 32-aligned).
            vref_e = const.tile([P, NJT, P], f32r, tag="vref_e")
            vself_e = const.tile([P, NJT, P], f32r, tag="vself_e")
            WoT = const.tile([P, C + P], f32r, tag="WoT")
            bias_row = const.tile([P, C], f32r, tag="bias_row")
            bias_bc = const.tile([P, C], f32, tag="bias_bc")
            onesK = const.tile([P, P], f32r, tag="onesK")
            blendw = const.tile([P, 2 * D], f32r, tag="blendw")
            recips = const.tile([P, 4 * IC], f32r, tag="recips")
            mergedT = const.tile([P, N], f32r, tag="mergedT")
            ones07 = blendw[:, 0:D]
            ones03 = blendw[:, D : 2 * D]

            # zero/one fills, kept minimal and disjoint from the DMA'd
            # regions so loads never wait on them (partition starts must be
            # 32-aligned, hence [64:P] covers the D:P padding).  Split across
            # DVE (idle early) and gpsimd so they don't serialize.
            for t in (qiT, klT, krT, kiT, qcT):
                nc.gpsimd.memset(t[64:P, :].bitcast(f32), 0.0)
            # zero-pad WvhT/WoT free dims so their matmuls hit the
            # fp32r fast path (1 cyc/row needs out free dim >= 256)
            nc.gpsimd.memset(WvhT[:, :, D:256].bitcast(f32), 0.0)
            nc.gpsimd.memset(WoT[:, C : C + P].bitcast(f32), 0.0)
            nc.gpsimd.memset(vref_e[:, :, D:P].bitcast(f32), 1.0)
            nc.gpsimd.memset(vself_e[:, :, D:P].bitcast(f32), 1.0)
            nc.gpsimd.memset(onesK[:].bitcast(f32), 0.0)
            nc.gpsimd.memset(onesK[0:1, :].bitcast(f32), 1.0)
            nc.gpsimd.memset(bias_row[:].bitcast(f32), 0.0)
            nc.gpsimd.memset(WoT[64:P, :].bitcast(f32), 0.0)
            nc.gpsimd.memset(mergedT[64:P, :].bitcast(f32), 0.0)
            nc.gpsimd.memset(recips[0:96, :].bitcast(f32), 0.0)

            # ---- DMAs, issued in consumer-priority order (they drain the
            # rings roughly in issue order): prologue weights first, then the
            # per-i-chunk slices of everything the first main iterations need
            nc.sync.dma_start(
                WqhT[:], WqhT_d.ap().rearrange("(o p) d -> p o d", p=P)
            )
            xT_r = xT_d.ap().rearrange("(o p) n -> p o n", p=P)
            vref_r = vref_d.ap().rearrange("(t p) d -> p t d", p=P)
            for ic in range(NICH):
                for c in range(NCT):
                    nc.sync.dma_start(
                        xT[:, c, ts(ic, IC)], xT_r[:, c, ts(ic, IC)]
                    )
                if ic == 0:
                    nc.sync.dma_start(
                        WvhT[:, :, 0:D],
                        WvhT_d.ap().rearrange("(o p) d -> p o d", p=P),
                    )
                for t, dram in (
                    (qiT, qiT_d), (klT, klT_d), (krT, krT_d), (kiT, kiT_d)
                ):
                    nc.sync.dma_start(
                        t[0:D, ts(ic, IC)], dram.ap()[:, ts(ic, IC)]
                    )
                nc.sync.dma_start(
                    vref_e[:, 4 * ic : 4 * ic + 4, 0:D],
                    vref_r[:, 4 * ic : 4 * ic + 4, :],
                )

                if ic == 0:
                    nc.sync.dma_start(blendw[:], blendw_d.ap())
            nc.sync.dma_start(bias_row[0:1, :], bias_d.ap()[None, :])
            nc.sync.dma_start(WoT[0:D, 0:C], WoT_d.ap())

            # ---- prologue: qcT / v_self projections, bias broadcast ----
            with tc.tile_pool(name="psum_pre", bufs=1, space="PSUM") as pp:
                # qcT: c-outer so the first matmuls only need xT c-tile 0
                qps = [
                    pp.tile([D, IC], f32, tag=f"qc{ic}", name=f"qc{ic}")
                    for ic in range(NICH)
                ]
                for c in range(NCT):
                    for ic in range(NICH):
                        nc.tensor.matmul(
                            qps[ic][:],
                            WqhT[:, c, :],
                            xT[:, c, ts(ic, IC)],
                            start=(c == 0),
                            stop=(c == NCT - 1),
                        )
                for ic in range(NICH):
                    # fold the (1-GAMMA)/GAMMA logit ratio into qcT here
                    nc.vector.tensor_scalar_mul(
                        qcT[0:D, ts(ic, IC)], qps[ic][:], (1.0 - GAMMA) / GAMMA
                    )
                # bias broadcast to all partitions via ones-row matmul
                bias_ps = pp.tile([P, C], f32, tag="bias")
                nc.tensor.matmul(
                    bias_ps[:, 0:IC], onesK[:], bias_row[:, 0:IC],
                    start=True, stop=True,
                )
                nc.tensor.matmul(
                    bias_ps[:, IC:C], onesK[:], bias_row[:, IC:C],
                    start=True, stop=True,
                )
                nc.vector.tensor_copy(bias_bc[:], bias_ps[:])
                for t in range(NJT):
                    psv = pp.tile([P, 256], f32, tag="vs", bufs=2)
                    for c in range(NCT):
                        nc.tensor.matmul(
                            psv[:],
                            xT[:, c, ts(t, P)],
                            WvhT[:, c, :],
                            start=(c == 0),
                            stop=(c == NCT - 1),
                        )
                    nc.vector.tensor_copy(vself_e[:, t, 0:D], psv[:, 0:D])

            # ---- main attention loop ----
            with tc.tile_pool(name="psum_main", bufs=2, space="PSUM") as pm:
                pending_blend = None

                def blend_pre(ic, outD, outS, c0=0, cw=IC):
                    # pull 1/Z out early so the deferred broadcast matmuls
                    # never wait on the DVE at an i-chunk boundary; chunk
                    # parity picks a disjoint recips region (no cross-chunk
                    # write-after-write on the tile)
                    o = 2 * (ic % 2) * IC
                    nc.vector.tensor_copy(
                        recips[96:P, o + c0 : o + c0 + cw], outD[96:P, c0 : c0 + cw]
                    )
                    nc.vector.tensor_copy(
                        recips[96:P, o + IC + c0 : o + IC + c0 + cw],
                        outS[96:P, c0 : c0 + cw],
                    )
                    with nc.allow_low_precision(reason="softmax denominator"):
                        nc.vector.reciprocal(
                            recips[96:P, o + c0 : o + c0 + cw],
                            recips[96:P, o + c0 : o + c0 + cw],
                        )
                        nc.vector.reciprocal(
                            recips[96:P, o + IC + c0 : o + IC + c0 + cw],
                            recips[96:P, o + IC + c0 : o + IC + c0 + cw],
                        )

                def blend(ic, outD, outS, c0=0, cw=IC):
                    # mergedT[:, window] = GAMMA*outD/Z_D + BETA*outS/Z_S
                    o = 2 * (ic % 2) * IC
                    bcD = pm.tile([P, IC], f32, tag="mixed", name="bcD", bufs=2)
                    nc.tensor.matmul(
                        bcD[0:D, c0 : c0 + cw], ones07,
                        recips[:, o + c0 : o + c0 + cw],
                        start=True, stop=True,
                    )
                    bcS = pm.tile([P, IC], f32, tag="self", name="bcS", bufs=2)
                    nc.tensor.matmul(
                        bcS[0:D, c0 : c0 + cw], ones03,
                        recips[:, o + IC + c0 : o + IC + c0 + cw],
                        start=True, stop=True,
                    )
                    last = ic == NICH - 1
                    oDs = work.tile([P, IC], f32, tag="em", bufs=4)
                    oSs = work.tile([P, IC], f32, tag="es", bufs=4)
                    if last:
                        # tail: use ACT so the copies run alongside DVE work
                        nc.scalar.copy(
                            oDs[0:D, c0 : c0 + cw], outD[0:D, c0 : c0 + cw]
                        )
                        nc.scalar.copy(
                            oSs[0:D, c0 : c0 + cw], outS[0:D, c0 : c0 + cw]
                        )
                    else:
                        # mid-loop: keep ACT free for the exps (cadence limiter)
                        nc.vector.tensor_copy(
                            oDs[0:D, c0 : c0 + cw], outD[0:D, c0 : c0 + cw]
                        )
                        nc.vector.tensor_copy(
                            oSs[0:D, c0 : c0 + cw], outS[0:D, c0 : c0 + cw]
                        )
                    w0 = ic * IC + c0
                    nc.vector.tensor_mul(
                        mergedT[0:D, w0 : w0 + cw],
                        oDs[0:D, c0 : c0 + cw],
                        bcD[0:D, c0 : c0 + cw],
                    )
                    m2 = work.tile([P, IC], f32, tag="m2")
                    nc.vector.tensor_mul(
                        m2[0:D, c0 : c0 + cw],
                        oSs[0:D, c0 : c0 + cw],
                        bcS[0:D, c0 : c0 + cw],
                    )
                    nc.vector.tensor_add(
                        mergedT[0:D, w0 : w0 + cw],
                        mergedT[0:D, w0 : w0 + cw],
                        m2[0:D, c0 : c0 + cw],
                    )
                def project(ic, c0=0, cw=IC):
                    # project a chunk window through Wout (n-tile t only needs
                    # mergedT columns from chunk t//4); psum slots are
                    # borrowed from the outD/outS tags
                    for t in range(4 * ic + c0 // P, 4 * ic + (c0 + cw) // P):
                        fin1 = pm.tile([P, IC], f32, tag="outD", name="fin1", bufs=2)
                        nc.tensor.matmul(
                            fin1[:], mergedT[:, ts(t, P)], WoT[:, 0:IC],
                            start=True, stop=True,
                        )
                        fin2 = pm.tile([P, 256], f32, tag="outS", name="fin2", bufs=2)
                        nc.tensor.matmul(
                            fin2[:], mergedT[:, ts(t, P)], WoT[:, IC : IC + 256],
                            start=True, stop=True,
                        )
                        fsb = fout.tile([P, C], f32, tag="fsb")
                        nc.vector.tensor_add(
                            fsb[:, 0:IC], bias_bc[:, 0:IC], fin1[:]
                        )
                        nc.vector.tensor_add(
                            fsb[:, IC:C], bias_bc[:, IC:C], fin2[:, 0:P]
                        )
                        nc.sync.dma_start(
                            out_d.ap()[t * P : (t + 1) * P, :], fsb[:]
                        )

                for ic in range(NICH):
                    outD = pm.tile([P, IC], f32, tag="outD", bufs=2)
                    outS = pm.tile([P, IC], f32, tag="outS", bufs=2)
                    for j in range(NJT):
                        ps_m = pm.tile([P, IC], f32, tag="mixed", bufs=2)
                        nc.tensor.matmul(
                            ps_m[:],
                            klT[:, ts(j, P)],
                            qiT[:, ts(ic, IC)],
                            start=True,
                            stop=False,
                        )
                        nc.tensor.matmul(
                            ps_m[:],
                            krT[:, ts(j, P)],
                            qcT[:, ts(ic, IC)],
                            start=False,
                            stop=True,
                        )
                        ps_s = pm.tile([P, IC], f32, tag="self", bufs=2)
                        nc.tensor.matmul(
                            ps_s[:],
                            kiT[:, ts(j, P)],
                            qiT[:, ts(ic, IC)],
                            start=True,
                            stop=True,
                        )
                        em = work.tile([P, IC], f32r, tag="em", bufs=4)
                        nc.scalar.activation(em[:], ps_m[:], Exp, scale=GAMMA * SCALE)
                        es = work.tile([P, IC], f32r, tag="es", bufs=4)
                        nc.scalar.activation(es[:], ps_s[:], Exp, scale=SCALE)
                        nc.tensor.matmul(
                            outD[:],
                            vref_e[:, j, :],
                            em[:],
                            start=(j == 0),
                            stop=(j == NJT - 1),
                        )
                        nc.tensor.matmul(
                            outS[:],
                            vself_e[:, j, :],
                            es[:],
                            start=(j == 0),
                            stop=(j == NJT - 1),
                        )
                        if j == 1 and pending_blend is not None:
                            # deferred: keeps the PE queue from stalling on the
                            # DVE reciprocal at the i-chunk boundary
                            blend(*pending_blend)
                        if j == 3 and pending_blend is not None:
                            # projection two iterations later still: by now the
                            # blend evac copies have released the outD/outS
                            # slots, so fin1/fin2 hold them only briefly
                            project(pending_blend[0])
                            pending_blend = None
                    if ic < NICH - 1:
                        blend_pre(ic, outD, outS)
                        pending_blend = (ic, outD, outS)
                    else:
                        # final chunk: two half-width rounds so the tail
                        # blend/projection chain pipelines
                        blend_pre(ic, outD, outS, 0, IC // 2)
                        blend(ic, outD, outS, 0, IC // 2)
                        project(ic, 0, IC // 2)
                        blend_pre(ic, outD, outS, IC // 2, IC // 2)
                        blend(ic, outD, outS, IC // 2, IC // 2)
                        project(ic, IC // 2, IC // 2)

    nc.compile()
    return nc


def _get_nc():
    if "nc" not in _CACHE:
        _CACHE["nc"] = _build_nc()
    return _CACHE["nc"]


def kernel(x, q_inj, k_inj, k_ref, k_refL, v_ref, Wq, Wv, Wout, bout):
    global LAST_EXEC_NS
    f = np.float32
    x = np.asarray(x, f)
    q_inj = np.asarray(q_inj, f)
    k_inj = np.asarray(k_inj, f)
    k_ref = np.asarray(k_ref, f)
    k_refL = np.asarray(k_refL, f)
    v_ref = np.asarray(v_ref, f)
    Wq = np.asarray(Wq, f)
    Wv = np.asarray(Wv, f)
    Wout = np.asarray(Wout, f)
    bout = np.asarray(bout, f)

    nc = _get_nc()
    xT = np.ascontiguousarray(x[0].T)
    blendw = np.zeros((P, 2 * D), f)
    blendw[96, 0:D] = GAMMA       # picks 1/Z_D (dup'd on partitions 96:128)
    blendw[96, D : 2 * D] = BETA  # picks 1/Z_S
    in_maps = []
    for h in range(NCORES):
        sl = slice(h * D, (h + 1) * D)
        in_maps.append(
            {
                "xT": xT,
                "qiT": np.ascontiguousarray(q_inj[h].T),
                "kiT": np.ascontiguousarray(k_inj[h].T),
                "krT": np.ascontiguousarray(k_ref[h].T),
                "klT": np.ascontiguousarray(k_refL[h].T),
                "vref": np.ascontiguousarray(v_ref[h]),
                "WqhT": np.ascontiguousarray(Wq[sl, :].T),
                "WvhT": np.ascontiguousarray(Wv[sl, :].T),
                "WoT": np.ascontiguousarray(Wout[:, sl].T),
                "bias": np.ascontiguousarray(bout if h == 0 else np.zeros(C, f)),
                "blendw": blendw,
            }
        )

    from concourse.bass_utils import run_bass_kernel_spmd

    trace = bool(os.environ.get("TRN_TRACE"))
    try:
        res = run_bass_kernel_spmd(
            nc, in_maps, core_ids=list(range(NCORES)), trace=trace
        )
    except ModuleNotFoundError:
        # axon NTFF profiling hook unavailable in this container
        res = run_bass_kernel_spmd(
            nc, in_maps, core_ids=list(range(NCORES)), trace=False
        )
    LAST_EXEC_NS = res.exec_time_ns
    out = np.zeros((N, C), f)
    for r in res.results:
        out += r["out"]
    return out.reshape(1, N, C)

